# revision 4
# baseline (speedup 1.0000x reference)
"""Trainium2 Bass kernel for nn_DiagLRConv (diag-embedded 5x5 conv, pad=2).

Math: out[n,o,h,w] = sum_{i,d} w[o,i,d] * xp[n,i,h+d,w+d]
(xp = x zero-padded by 2 in h and w; a diag_embed'ed 5x5 kernel is 5
diagonal taps mixed through a 16x16 channel matrix).

Mapping (per NeuronCore, 2 images each, 8 cores data-parallel over batch).
PE matmuls on this stack serialize (no subarray-tile concurrency credit),
so the kernel maximizes contraction K per matmul: only 2 matmuls per
output row instead of 5 (or the old 6 half-width ones):

  - SBUF per 64-row group: xq [96, 68, 518] fp16 built from three
    diagonally-shifted views of the same HBM rows:
      partitions  0..32 = s0 = [img0 ch; img1 ch]  rows 64g..,  cols 0..
      partitions 32..64 = s1 = same, shifted (+1 row, +1 col)  [2nd read]
      partitions 64..96 = s2 = shifted (+2,+2), one aligned 4x DVE copy
  - Round A: K=96 matmul, stationary [s-block b -> w(:, :, b)] block-diag
    over the two images -> taps 0,1,2 of both images in one matmul.
    Round B: K=64 on [s0; s1] at offset (+3 rows, +3 cols) -> taps 3,4.
    Both accumulate in one PSUM bank region [32e..32e+32, 512] where
    e = output row within a 4-row supertile (tile_position=(0, 32e)).
  - Weight-residency blocks: matmuls are issued in blocks of 4 supertiles
    as [4x ldweights(A); 16x matmul(ldweights=False); 4x ldweights(B);
    16x matmul(ldweights=False)] so 16 consecutive matmuls share one
    stationary set instead of alternating A/B reloads every matmul
    (measured ~70us/rep faster on HW than per-matmul self-loading).
  - ScalarE evacuates [128,512] fp32 -> fp16 per supertile; one 2 MB DMA
    per group on the ACT HWDGE queue (input prefetches own the SP queue)
    writes a kernel-native DRAM layout reassembled on host.
"""

import numpy as np

F16 = np.float16

_COMPILED = {}


def _trace_nc(H, reps=1, no_in=False, no_s2=False, no_mm=False, no_evac=False,
              no_out=False, ab_order=False, out_split=False):
    import concourse.mybir as mybir
    import concourse.tile as tile
    from concourse import bacc

    F32 = mybir.dt.float32
    FP16 = mybir.dt.float16

    assert H % 64 == 0
    G = H // 64              # 64-output-row groups
    RB = 68                  # input rows buffered per group (64 + 4 halo)
    WQ = 518                 # SBUF row width (516 used, padded even)

    nc = bacc.Bacc(None, target_bir_lowering=False, debug=False)
    xp = nc.declare_dram_parameter("xp", [2, 16, H + 5, WQ], FP16, isOutput=False)
    wd = nc.declare_dram_parameter("wd", [96, 2, 32], FP16, isOutput=False)
    # kernel-native output layout: [p=32e+16m+o, g, s, w], h = 64g + 4s + e
    y = nc.declare_dram_parameter("y", [128, G, 16, 512], FP16, isOutput=True)

    with tile.TileContext(nc) as tc:
        with (
            tc.tile_pool(name="const", bufs=1) as const,
            tc.tile_pool(name="xpool", bufs=2) as xpool,
            tc.tile_pool(name="psum", bufs=2, space="PSUM") as psum,
            tc.tile_pool(name="stpool", bufs=2) as stpool,
        ):
            wt = const.tile([96, 2, 32], FP16)
            nc.sync.dma_start(out=wt[:], in_=wd[:])

            groups = [(rep, g) for rep in range(reps) for g in range(G)]
            xqs = {}

            def load_group(idx):
                """Input DMAs (SP queue) + s2 DVE copy for group idx.

                The first group is loaded in row-chunks so the first
                supertiles' matmuls start ~10us earlier; later groups are
                fully hidden behind the previous group's matmul burst.
                """
                rep, g = groups[idx]
                xq = xpool.tile([96, RB, WQ], FP16, tag="xq",
                                name=f"xq{rep}_{g}")
                xqs[idx] = xq
                chunks = [(0, 26), (26, RB)] if idx == 0 else [(0, RB)]
                for lo, hi in chunks:
                    if not no_in:
                        for m in range(2):
                            # s0: [img0; img1] channels, rows 64g+lo..64g+hi
                            nc.sync.dma_start(
                                out=xq[16 * m : 16 * m + 16, lo:hi],
                                in_=xp[m, :, 64 * g + lo : 64 * g + hi, :],
                            )
                            # s1: shifted (+1 row, +1 col) second read
                            nc.sync.dma_start(
                                out=xq[32 + 16 * m : 48 + 16 * m, lo:hi, 0:516],
                                in_=xp[m, :, 64 * g + 1 + lo : 64 * g + 1 + hi,
                                       1:517],
                            )
                    elif idx == 0 and lo == 0:
                        nc.any.memset(xq[:], 0.25)
                    if not no_s2:
                        # s2 = s0 shifted (+2,+2): aligned DVE 4x copy over
                        # the dst rows whose src rows this chunk completes
                        c0, c1 = max(0, lo - 2), hi - 2
                        nc.vector.tensor_copy(
                            xq[64:96, c0:c1, 0:516],
                            xq[0:32, c0 + 2 : c1 + 2, 2:518],
                        )

            load_group(0)
            for idx, (rep, g) in enumerate(groups):
                if idx + 1 < len(groups):
                    load_group(idx + 1)   # prefetch ahead of this group's MMs
                xq = xqs.pop(idx)

                st = stpool.tile([128, 16, 512], FP16, tag="st",
                                 name=f"st{rep}_{g}")

                def mm_noload(out, lhsT, rhs, start, stop, tile_position):
                    """InstMatmult with ldweights=False: reuses the stationary
                    loaded by a preceding standalone nc.tensor.ldweights()
                    (fp16 path; avoids the per-matmul drain+reload)."""
                    te = nc.tensor
                    ifmap_ap = te.lower_ap(rhs.opt({0}), opt=False)
                    weights_ap = te.lower_ap(
                        lhsT.opt({0}), opt=False, for_matmul_weights=True
                    )
                    out_ap = te.lower_ap(out)

                    def rup(sz):
                        for v in (32, 64, 128):
                            if v >= sz:
                                return v
                        raise AssertionError(sz)

                    te.add_instruction(
                        mybir.InstMatmult(
                            name=te.bass.get_next_instruction_name(),
                            replication_resolution=0,
                            replication_shift_amnt=0,
                            replication_num_rows=0,
                            start_tensor_calc=start,
                            stop_tensor_calc=stop,
                            ins=[ifmap_ap, weights_ap],
                            outs=[out_ap],
                            perf_mode=None,
                            is_transpose=None,
                            ifmap_quant_offset=None,
                            weights_quant_offset=None,
                            bass_skip_group_check=True,
                            tile_position=tile_position,
                            tile_size=(
                                rup(rhs.partition_size()),
                                rup(out.partition_size()),
                            ),
                            ldweights=False,
                        )
                    )

                SB = 4                   # supertiles per weight-load block
                for blk in range(0 if no_mm else 16 // SB):
                    pss = [
                        psum.tile([128, 512], F32, tag=f"ps{q}",
                                  name=f"ps{rep}_{g}_{blk}_{q}")
                        for q in range(SB)
                    ]
                    # round A resident: taps 0,1,2 (K=96) at 4 col positions
                    for e in range(4):
                        nc.tensor.ldweights(
                            wt[0:96, 0, :], tile_position=(0, 32 * e)
                        )
                    for q in range(SB):
                        s = SB * blk + q
                        for e in range(4):
                            t = 4 * s + e
                            mm_noload(
                                pss[q][32 * e : 32 * e + 32, :],
                                wt[0:96, 0, :],
                                xq[0:96, t, 0:512],
                                start=True, stop=False,
                                tile_position=(0, 32 * e),
                            )
                    # round B resident: taps 3,4 (K=64)
                    for e in range(4):
                        nc.tensor.ldweights(
                            wt[0:64, 1, :], tile_position=(0, 32 * e)
                        )
                    for q in range(SB):
                        s = SB * blk + q
                        for e in range(4):
                            t = 4 * s + e
                            mm_noload(
                                pss[q][32 * e : 32 * e + 32, :],
                                wt[0:64, 1, :],
                                xq[0:64, t + 3, 3:515],
                                start=False, stop=True,
                                tile_position=(0, 32 * e),
                            )
                    if not no_evac:
                        for q in range(SB):
                            nc.scalar.copy(st[:, SB * blk + q, :], pss[q][:])
                if no_mm or no_evac:
                    nc.any.memset(st[:], 0.0)
                if not no_out:
                    # single out-DMA per group on the ACT HWDGE queue (never
                    # blocks SP input prefetches; extra per-DMA fixed cost on
                    # the ACT ring measurably stalls evacs if split finer)
                    lo = 12 if (out_split and not no_mm and not no_evac) else 0
                    nc.scalar.dma_start(out=y[:, g, lo:16], in_=st[:, lo:16])
    nc.compile()
    return nc


def _get_nc(H, reps=1, **kw):
    key = (H, reps, tuple(sorted(kw.items())))
    if key not in _COMPILED:
        _COMPILED[key] = _trace_nc(H, reps, **kw)
    return _COMPILED[key]


def _prep_inputs(x, filter_w, H):
    """x: [N,16,H,512] fp32, filter_w: [16,16,5] fp32 -> per-core in_maps."""
    N = x.shape[0]
    n_cores = N // 2
    x16 = x.astype(F16)

    w16 = filter_w.astype(F16)
    wT = np.transpose(w16, (1, 2, 0))   # [i, d, o]
    wd = np.zeros((96, 2, 32), dtype=F16)
    for b in range(3):                  # round A: taps 0,1,2 on s-block b
        for m in range(2):
            wd[32 * b + 16 * m : 32 * b + 16 * m + 16, 0,
               16 * m : 16 * m + 16] = wT[:, b, :]
    for b in range(2):                  # round B: taps 3,4 on s-blocks 0,1
        for m in range(2):
            wd[32 * b + 16 * m : 32 * b + 16 * m + 16, 1,
               16 * m : 16 * m + 16] = wT[:, 3 + b, :]
    in_maps = []
    for cid in range(n_cores):
        xprep = np.zeros((2, 16, H + 5, 518), dtype=F16)
        xprep[:, :, 2 : H + 2, 2:514] = x16[2 * cid : 2 * cid + 2]
        in_maps.append({"xp": xprep, "wd": wd})
    return in_maps


def _reassemble(yk, H):
    # yk [128, G, 16, 512] fp16; p = 32e+16m+o, h = 64g + 4s + e
    G = H // 64
    y6 = yk.reshape(4, 2, 16, G, 16, 512)            # [e, m, o, g, s, w]
    out = np.transpose(y6, (1, 2, 3, 4, 0, 5))       # [m, o, g, s, e, w]
    return np.ascontiguousarray(out).reshape(2, 16, H, 512).astype(np.float32)


def kernel(x, filter_w):
    from concourse.bass_utils import run_bass_kernel_spmd

    x = np.asarray(x)
    filter_w = np.asarray(filter_w)
    N, C, H, W = x.shape
    assert (C, W) == (16, 512) and N % 2 == 0

    nc = _get_nc(H)
    in_maps = _prep_inputs(x, filter_w, H)
    n_cores = len(in_maps)
    res = run_bass_kernel_spmd(nc, in_maps, list(range(n_cores)))
    out = np.empty((N, 16, H, 512), dtype=np.float32)
    for cid in range(n_cores):
        out[2 * cid : 2 * cid + 2] = _reassemble(res.results[cid]["y"], H)
    return out


if __name__ == "__main__":
    import sys
    H = int(sys.argv[1]) if len(sys.argv) > 1 else 64
    rng = np.random.default_rng(0)
    x = rng.standard_normal((16, 16, H, 512)).astype(np.float32)
    fw = (rng.standard_normal((16, 16, 5)) * 0.1).astype(np.float32)
    out = kernel(x, fw)

    xpad = np.zeros((16, 16, H + 4, 516), dtype=np.float64)
    xpad[:, :, 2 : H + 2, 2:514] = x
    ref = np.zeros_like(out, dtype=np.float64)
    for k in range(5):
        sh = xpad[:, :, k : k + H, k : k + 512]
        ref += np.einsum("oik,nihw->nohw", fw[:, :, k : k + 1].astype(np.float64), sh)
    rel = np.linalg.norm(out - ref) / np.linalg.norm(ref)
    mx = np.abs(out - ref).max() / np.abs(ref).max()
    print(f"self-test H={H}: rel l2 err {rel:.3e}, max err {mx:.3e}")


# revision 6
# speedup vs baseline: 1.9201x; 1.9201x over previous
"""Trainium2 Bass kernel for nn_DiagLRConv (diag-embedded 5x5 conv, pad=2).

Math: out[n,o,h,w] = sum_{i,d} w[o,i,d] * xp[n,i,h+d,w+d]
(xp = x zero-padded by 2 in h and w).

Mapping: spatial (H) sharding -- each of the 8 cores computes output rows
[64c, 64c+64) of ALL 16 images. SBUF partitions hold (image, channel)
pairs, so the diagonal tap shift (+d, +d) is a pure free-dim AP offset:
no shifted copies, no second HBM read, no DVE plane copies.

  - xq per octet: [128, 68, 518] fp16, partition p = 16j+i holds image
    j(+8*o8) channel i, rows 64c..64c+68 of the padded input.
  - One matmul per (output row, tap, octet): K=128 (8 images x 16 ch),
    M=128 (8 images x 16 out-ch) block-diagonal stationary
    wd[16j+i, d, 16j'+o] = delta_jj' w[o,i,d]; moving
    xq[0:128, r+d, d:d+512]; 5 taps accumulate into one PSUM bank.
    => 2 octets x 64 rows x 5 taps = 640 full-array matmuls.
  - Weight-residency blocks of 4 rows: one PSUM tile [128,4,512]
    (4 banks), per tap one standalone ldweights (P=128 -> FWL) + 4
    no-load matmuls (InstMatmult(ldweights=False)).
  - ScalarE evacuates [128, 4x512] fp32 -> fp16 per block; one ~1 MB
    out-DMA per 2 blocks on the ACT HWDGE queue.
  - Input DMAs chunked by rows (SP queue), octet0 first so compute
    starts after ~7us; octet1 loads under octet0's matmul burst.
"""

import numpy as np

F16 = np.float16

_COMPILED = {}


def _trace_nc(H, reps=1, no_in=False, no_mm=False, no_evac=False,
              no_out=False):
    import concourse.mybir as mybir
    import concourse.tile as tile
    from concourse import bacc

    F32 = mybir.dt.float32
    FP16 = mybir.dt.float16

    assert H % 64 == 0
    R = 64                   # output rows per core
    RB = 68                  # input rows buffered (64 + 4 halo)
    WQ = 518                 # row width (516 used, padded even)

    nc = bacc.Bacc(None, target_bir_lowering=False, debug=False)
    # per-core H-shard: rows 64c..64c+68 of the padded input, all 16 images
    xp = nc.declare_dram_parameter("xp", [16, 16, RB, WQ], FP16,
                                   isOutput=False)
    wd = nc.declare_dram_parameter("wd", [128, 5, 128], FP16, isOutput=False)
    # output: y[p=16j+o, o8, blk2, s8, w], out row r = 8*blk2 + s8
    y = nc.declare_dram_parameter("y", [128, 2, 8, 8, 512], F16 if False
                                  else FP16, isOutput=True)

    with tile.TileContext(nc) as tc:
        with (
            tc.tile_pool(name="const", bufs=1) as const,
            tc.tile_pool(name="xpool", bufs=1) as xpool,
            tc.tile_pool(name="psum", bufs=2, space="PSUM") as psum,
            tc.tile_pool(name="stpool", bufs=2) as stpool,
        ):
            wt = const.tile([128, 5, 128], FP16)
            nc.sync.dma_start(out=wt[:], in_=wd[:])

            def mm_noload(out, lhsT, rhs, start, stop):
                """Full-array InstMatmult with ldweights=False (reuses the
                stationary loaded by a standalone nc.tensor.ldweights)."""
                te = nc.tensor
                ifmap_ap = te.lower_ap(rhs.opt({0}), opt=False)
                weights_ap = te.lower_ap(
                    lhsT.opt({0}), opt=False, for_matmul_weights=True
                )
                out_ap = te.lower_ap(out)
                te.add_instruction(
                    mybir.InstMatmult(
                        name=te.bass.get_next_instruction_name(),
                        replication_resolution=0,
                        replication_shift_amnt=0,
                        replication_num_rows=0,
                        start_tensor_calc=start,
                        stop_tensor_calc=stop,
                        ins=[ifmap_ap, weights_ap],
                        outs=[out_ap],
                        perf_mode=None,
                        is_transpose=None,
                        ifmap_quant_offset=None,
                        weights_quant_offset=None,
                        bass_skip_group_check=True,
                        tile_position=(0, 0),
                        tile_size=(128, 128),
                        ldweights=False,
                    )
                )

            CHUNKS = [(0, 18), (18, 36), (36, 52), (52, RB)]
            xqs = {}

            def load_octet(rep, o8):
                xq = xpool.tile([128, RB, WQ], FP16, tag=f"xq{o8}",
                                name=f"xq{rep}_{o8}")
                xqs[(rep, o8)] = xq
                for lo, hi in CHUNKS:
                    if no_in:
                        if rep == 0 and lo == 0:
                            nc.any.memset(xq[:], 0.25)
                        continue
                    for j in range(8):   # image within octet
                        nc.sync.dma_start(
                            out=xq[16 * j : 16 * j + 16, lo:hi],
                            in_=xp[8 * o8 + j, :, lo:hi, :],
                        )

            load_octet(0, 0)
            load_octet(0, 1)
            for rep in range(reps):
                if rep + 1 < reps:
                    pass  # next rep's loads are issued per-octet below
                for o8 in range(2):
                    xq = xqs.pop((rep, o8))
                    last = rep == reps - 1 and o8 == 1
                    st = None
                    for blk in range(0 if no_mm else 16):   # 4 rows per blk
                        if blk % 2 == 0:
                            st = stpool.tile([128, 8, 512], FP16, tag="st",
                                             name=f"st{rep}_{o8}_{blk}")
                        ps = psum.tile([128, 4, 512], F32, tag="ps",
                                       name=f"ps{rep}_{o8}_{blk}")
                        for d in range(5):
                            nc.tensor.ldweights(
                                wt[:, d, :], tile_position=(0, 0)
                            )
                            for q in range(4):
                                r = 4 * blk + q
                                mm_noload(
                                    ps[:, q, :],
                                    wt[:, d, :],
                                    xq[:, r + d, d : d + 512],
                                    start=(d == 0),
                                    stop=(d == 4),
                                )
                        if not no_evac:
                            b1 = blk % 2
                            nc.scalar.copy(st[:, 4 * b1 : 4 * b1 + 4, :],
                                           ps[:])
                        if last and blk == 14 and not no_out and not no_mm \
                                and not no_evac:
                            # drain the final st's first half early so only
                            # a half-DMA remains after the last evac
                            nc.scalar.dma_start(
                                out=y[:, o8, 7, 0:4], in_=st[:, 0:4]
                            )
                        if blk % 2 == 1 and not no_out and not no_mm \
                                and not no_evac:
                            if last and blk == 15:
                                nc.scalar.dma_start(
                                    out=y[:, o8, 7, 4:8], in_=st[:, 4:8]
                                )
                            else:
                                nc.scalar.dma_start(
                                    out=y[:, o8, blk // 2], in_=st[:]
                                )
                        if blk == 3 and rep + 1 < reps and o8 == 1:
                            # prefetch next rep's octets under this burst
                            load_octet(rep + 1, 0)
                            load_octet(rep + 1, 1)
                    if no_mm or no_evac:
                        stz = stpool.tile([128, 8, 512], FP16, tag="st",
                                          name=f"stz{rep}_{o8}")
                        nc.any.memset(stz[:], 0.0)
                        if not no_out:
                            for b2 in range(8):
                                nc.scalar.dma_start(out=y[:, o8, b2],
                                                    in_=stz[:])
    nc.compile()
    return nc


def _get_nc(H, reps=1, **kw):
    key = (H, reps, tuple(sorted(kw.items())))
    if key not in _COMPILED:
        _COMPILED[key] = _trace_nc(H, reps, **kw)
    return _COMPILED[key]


def _prep_inputs(x, filter_w, H):
    """x: [16,16,H,512] fp32, filter_w: [16,16,5] fp32 -> per-core in_maps.

    Spatial sharding: core c gets padded rows 64c..64c+68 of all images.
    """
    N = x.shape[0]
    n_cores = H // 64
    x16 = x.astype(F16)

    w16 = filter_w.astype(F16)
    wT = np.transpose(w16, (1, 2, 0))   # [i, d, o]
    wd = np.zeros((128, 5, 128), dtype=F16)
    for j in range(8):
        wd[16 * j : 16 * j + 16, :, 16 * j : 16 * j + 16] = wT
    xpf = np.zeros((N, 16, H + 5, 518), dtype=F16)
    xpf[:, :, 2 : H + 2, 2:514] = x16
    in_maps = []
    for c in range(n_cores):
        in_maps.append(
            {"xp": np.ascontiguousarray(xpf[:, :, 64 * c : 64 * c + 68, :]),
             "wd": wd}
        )
    return in_maps


def _reassemble(yk):
    # yk [128, 2, 8, 8, 512] fp16; p = 16j+o, r = 8*blk2 + s8
    # -> out rows [16 imgs, 16 ch, 64, 512] fp32 for this core's row range
    y6 = yk.reshape(8, 16, 2, 8, 8, 512)             # [j, o, o8, b2, s8, w]
    out = np.transpose(y6, (2, 0, 1, 3, 4, 5))       # [o8, j, o, b2, s8, w]
    return np.ascontiguousarray(out).reshape(16, 16, 64, 512).astype(
        np.float32)


def kernel(x, filter_w):
    from concourse.bass_utils import run_bass_kernel_spmd

    x = np.asarray(x)
    filter_w = np.asarray(filter_w)
    N, C, H, W = x.shape
    assert (C, W) == (16, 512) and N == 16 and H % 64 == 0

    nc = _get_nc(H)
    in_maps = _prep_inputs(x, filter_w, H)
    n_cores = len(in_maps)
    res = run_bass_kernel_spmd(nc, in_maps, list(range(n_cores)))
    out = np.empty((N, 16, H, 512), dtype=np.float32)
    for c in range(n_cores):
        out[:, :, 64 * c : 64 * c + 64, :] = _reassemble(res.results[c]["y"])
    return out


if __name__ == "__main__":
    import sys
    H = int(sys.argv[1]) if len(sys.argv) > 1 else 512
    rng = np.random.default_rng(0)
    x = rng.standard_normal((16, 16, H, 512)).astype(np.float32)
    fw = (rng.standard_normal((16, 16, 5)) * 0.1).astype(np.float32)
    out = kernel(x, fw)

    xpad = np.zeros((16, 16, H + 4, 516), dtype=np.float64)
    xpad[:, :, 2 : H + 2, 2:514] = x
    ref = np.zeros_like(out, dtype=np.float64)
    for k in range(5):
        sh = xpad[:, :, k : k + H, k : k + 512]
        ref += np.einsum("oik,nihw->nohw", fw[:, :, k : k + 1].astype(np.float64), sh)
    rel = np.linalg.norm(out - ref) / np.linalg.norm(ref)
    mx = np.abs(out - ref).max() / np.abs(ref).max()
    print(f"self-test H={H}: rel l2 err {rel:.3e}, max err {mx:.3e}")
